# revision 1
# baseline (speedup 1.0000x reference)
"""Trainium2 Bass kernel: CrossAttention (B=2, Nq=1024, Nkv=2048, D=1024, H=16).

Sharding: 8 cores = 2 (batch) x 4 (head groups of 4 heads).
Each core computes, for its batch b and heads [4g, 4g+4):
    qT = (x_b @ Wq_s)^T            [256, 1024]   (dd on partitions)
    kT = (mem_b @ Wk_s)^T          [256, 2048]
    v  = mem_b @ Wv_s              [2048, 256]   (+ ones column per head)
    per head: sT = k_h @ q_h^T     [2048, 1024]  (j on partitions)
              eT = exp(SCALE*sT)
              cu = [v_h | 1]^T-acc [65, 1024]    (row 64 = softmax denom)
              ctx_h = cu[0:64] * recip(cu[64]) broadcast
    part = ctx @ Wp_s              [1024, 1024]  (row-parallel partial)
Host sums the 4 partials per batch and adds b_proj.

All matmuls run in float32r (fp32 rounded to 11-bit mantissa; full PE rate at
N>=256).  Inputs are pre-rounded on the host; on-device producers of matmul
operands write float32r directly (ACT/DVE convert on write).
"""

import numpy as np
import ml_dtypes

DIM = 1024
HEADS = 16
HD = 64
B = 2
NQ = 1024
NKV = 2048
SCALE = HD ** -0.5
N_CORES = 8
HG = 4               # heads per core
DD = HG * HD         # 256 packed head dims per core
KC = 8               # contraction chunks (DIM / 128)
JC = NKV // 128      # 16 kv-row chunks

_CACHE = {}


def round_fp32r(a):
    """Round fp32 to fp32r (11-bit mantissa, low 12 bits zero), RNE."""
    u = np.ascontiguousarray(a, np.float32).view(np.uint32)
    low = u & np.uint32(0xFFF)
    hi = u & np.uint32(0xFFFFF000)
    up = (low > 0x800) | ((low == 0x800) & (((hi >> 12) & 1) == 1))
    hi = hi + (up.astype(np.uint32) << 12)
    return hi.view(np.float32)


def _build_module():
    import concourse.bacc as bacc
    import concourse.tile as tile
    import concourse.mybir as mybir

    f32 = mybir.dt.float32
    f32r = mybir.dt.float32r
    bf16 = mybir.dt.bfloat16
    EXP = mybir.ActivationFunctionType.Exp

    nc = bacc.Bacc(
        trn_type="TRN2",
        target_bir_lowering=False,
        debug=False,
        num_devices=N_CORES,
    )

    xt_d = nc.dram_tensor("xt", [128, KC, NQ], f32r, kind="ExternalInput").ap()
    memt_d = nc.dram_tensor("memt", [128, KC, NKV], f32r, kind="ExternalInput").ap()
    wq_d = nc.dram_tensor("wq", [128, KC, DD], f32r, kind="ExternalInput").ap()
    wk_d = nc.dram_tensor("wk", [128, KC, DD], f32r, kind="ExternalInput").ap()
    wv_d = nc.dram_tensor("wv", [128, KC, DD], f32r, kind="ExternalInput").ap()
    wp_d = nc.dram_tensor("wp", [64, HG, DIM], bf16, kind="ExternalInput").ap()
    ones_d = nc.dram_tensor("ones_in", [1, HD], f32r, kind="ExternalInput").ap()
    vones_d = nc.dram_tensor(
        "vones", [128, JC * HG], f32r, kind="ExternalInput"
    ).ap()
    out_d = nc.dram_tensor("out", [NQ, DIM], f32, kind="ExternalOutput").ap()

    with tile.TileContext(nc) as tc:
        with (
            tc.tile_pool(name="wpool", bufs=1) as wpool,
            tc.tile_pool(name="persist", bufs=1) as persist,
            tc.tile_pool(name="xstream", bufs=2) as xstream,
            tc.tile_pool(name="work", bufs=2) as work,
            tc.tile_pool(name="opool", bufs=2) as opool,
            tc.tile_pool(name="psum", bufs=4, space="PSUM") as psum,
        ):
            # ---- weights / inputs ----
            # qT inputs first: the qT matmuls only need wq + one xt chunk, so
            # their DMAs go ahead of the 8 MB memt load in the queues.
            wq_sb = wpool.tile([128, KC, DD], f32r, name="wq_sb")
            nc.sync.dma_start(out=wq_sb, in_=wq_d)

            # ---- qT projection: qT[dd, i] = sum_k Wq[k, dd] * x[i, k] ----
            qT_sb = persist.tile([128, 2, NQ], bf16, name="qT_sb")
            qt_ps = [
                psum.tile([128, NQ], f32, name=f"qt_ps{mc}", tag="ps")
                for mc in range(2)
            ]
            for kc in range(KC):
                xt_sb = xstream.tile([128, NQ], f32r, name="xt_sb", tag="xt")
                nc.sync.dma_start(out=xt_sb, in_=xt_d[:, kc, :])
                for mc in range(2):
                    for ih in range(2):
                        nc.tensor.matmul(
                            qt_ps[mc][:, ih * 512 : (ih + 1) * 512],
                            lhsT=wq_sb[:, kc, mc * 128 : (mc + 1) * 128],
                            rhs=xt_sb[:, ih * 512 : (ih + 1) * 512],
                            start=(kc == 0),
                            stop=(kc == KC - 1),
                        )
            for mc in range(2):
                nc.vector.tensor_copy(out=qT_sb[:, mc, :], in_=qt_ps[mc])

            # remaining weights + memt
            wk_sb = wpool.tile([128, KC, DD], f32r, name="wk_sb")
            nc.sync.dma_start(out=wk_sb, in_=wk_d)
            wv_sb = wpool.tile([128, KC, DD], f32r, name="wv_sb")
            nc.sync.dma_start(out=wv_sb, in_=wv_d)
            wp_sb = wpool.tile([64, HG, DIM], bf16, name="wp_sb")
            nc.sync.dma_start(out=wp_sb, in_=wp_d)

            memt_sb = []
            for kc in range(KC):
                m = wpool.tile(
                    [128, NKV], f32r, name=f"memt_sb{kc}", tag=f"memt{kc}"
                )
                nc.sync.dma_start(out=m, in_=memt_d[:, kc, :])
                memt_sb.append(m)

            ones_sb = wpool.tile([65, HD], f32r, name="ones_sb")
            nc.sync.dma_start(out=ones_sb[64:65, :], in_=ones_d)
            vones_sb = wpool.tile([128, JC * HG], f32r, name="vones_sb")
            nc.sync.dma_start(out=vones_sb, in_=vones_d)

            # ---- persistent intermediates ----
            kT_sb = persist.tile([128, 2, NKV], bf16, name="kT_sb")
            vaug_sb = persist.tile([128, JC, HG, HD + 1], bf16, name="vaug_sb")
            ctx_sb = persist.tile([64, HG, NQ], bf16, name="ctx_sb")

            # ones column of vaug, via DVE copy from the DMA'd strip
            nc.vector.tensor_copy(
                out=vaug_sb[:, :, :, HD : HD + 1],
                in_=vones_sb.rearrange("p (j h) -> p j h", j=JC)[:, :, :, None],
            )

            # ---- kT projection: kT[dd, j] = sum_k Wk[k, dd] * mem[j, k] ----
            kt_ps = {}
            for mc in range(2):
                for jh2 in range(2):
                    kt_ps[mc, jh2] = psum.tile(
                        [128, NKV // 2], f32, name=f"kt_ps_{mc}_{jh2}", tag="ps"
                    )
            for kc in range(KC):
                for mc in range(2):
                    for jh2 in range(2):
                        for jh in range(2):
                            j0 = (jh2 * 2 + jh) * 512
                            nc.tensor.matmul(
                                kt_ps[mc, jh2][:, jh * 512 : (jh + 1) * 512],
                                lhsT=wk_sb[:, kc, mc * 128 : (mc + 1) * 128],
                                rhs=memt_sb[kc][:, j0 : j0 + 512],
                                start=(kc == 0),
                                stop=(kc == KC - 1),
                            )
            for mc in range(2):
                for jh2 in range(2):
                    nc.vector.tensor_copy(
                        out=kT_sb[:, mc, jh2 * 1024 : (jh2 + 1) * 1024],
                        in_=kt_ps[mc, jh2],
                    )

            # ---- v projection: v[j, dd] = sum_k mem[j, k] * Wv[k, dd] ----
            for jc in range(JC):
                v_ps = psum.tile([128, DD], f32, name=f"v_ps{jc}", tag="ps")
                for kc in range(KC):
                    nc.tensor.matmul(
                        v_ps,
                        lhsT=memt_sb[kc][:, jc * 128 : (jc + 1) * 128],
                        rhs=wv_sb[:, kc, :],
                        start=(kc == 0),
                        stop=(kc == KC - 1),
                    )
                nc.vector.tensor_copy(
                    out=vaug_sb[:, jc, :, 0:HD],
                    in_=v_ps.rearrange("p (h d) -> p h d", h=HG),
                )

            # ---- attention, head pairs (h0 at partitions 0-63, h1 at 64-127;
            # their K=64 QK matmuls run concurrently in different PE row groups)
            for hp in range(2):
                heads = (2 * hp, 2 * hp + 1)
                cu = {}
                for h in heads:
                    cu[h] = psum.tile(
                        [HD + 1, NQ], f32, name=f"cu_ps{h}", tag="ps"
                    )
                for jc in range(JC):
                    sT = {}
                    for h in heads:
                        po = (h % 2) * 64
                        sT[h] = psum.tile(
                            [128, NQ], f32, name=f"sT_ps_{h}_{jc}", tag="ps"
                        )
                        for ih in range(2):
                            nc.tensor.matmul(
                                sT[h][:, ih * 512 : (ih + 1) * 512],
                                lhsT=kT_sb[
                                    po : po + 64, hp, jc * 128 : (jc + 1) * 128
                                ],
                                rhs=qT_sb[
                                    po : po + 64, hp, ih * 512 : (ih + 1) * 512
                                ],
                                start=True,
                                stop=True,
                            )
                    for h in heads:
                        eT_sb = work.tile([128, NQ], bf16, name="eT_sb", tag="eT", bufs=4)
                        nc.scalar.activation(
                            out=eT_sb, in_=sT[h], func=EXP, scale=SCALE
                        )
                        for ih in range(2):
                            nc.tensor.matmul(
                                cu[h][:, ih * 512 : (ih + 1) * 512],
                                lhsT=vaug_sb[:, jc, h, :],
                                rhs=eT_sb[:, ih * 512 : (ih + 1) * 512],
                                start=(jc == 0),
                                stop=(jc == JC - 1),
                            )
                # softmax denom: 1/den = exp(-ln(den)) on ACT (DVE recip on a
                # single partition costs ~6.5us; two ACT passes cost ~2.3us)
                for h in heads:
                    # 1/den = exp(-ln(den)) on ACT (cheap); broadcast the
                    # row across 64 partitions on the idle GpSimd engine.
                    inv_sb = work.tile(
                        [65, NQ], f32r, name="inv_sb", tag="inv", bufs=2
                    )
                    nc.scalar.activation(
                        out=inv_sb[64:65, :],
                        in_=cu[h][64:65, :],
                        func=mybir.ActivationFunctionType.Ln,
                    )
                    nc.scalar.activation(
                        out=inv_sb[64:65, :],
                        in_=inv_sb[64:65, :],
                        func=EXP,
                        scale=-1.0,
                    )
                    bden_ps = psum.tile([64, NQ], f32, name=f"bden_ps{h}", tag="ps")
                    for ih in range(2):
                        nc.tensor.matmul(
                            bden_ps[:, ih * 512 : (ih + 1) * 512],
                            lhsT=ones_sb[64:65, :],
                            rhs=inv_sb[64:65, ih * 512 : (ih + 1) * 512],
                            start=True,
                            stop=True,
                        )
                    bden_sb = work.tile(
                        [64, NQ], f32, name="bden_sb", tag="bden", bufs=2
                    )
                    nc.vector.tensor_copy(out=bden_sb, in_=bden_ps)
                    nc.vector.tensor_mul(ctx_sb[:, h, :], cu[h][0:HD, :], bden_sb)

            # ---- output projection: part[i, n] = sum_h sum_d ctx[d,h,i] Wp[d,h,n]
            for ic in range(8):
                pr_ps = psum.tile([128, DIM], f32, name=f"pr_ps{ic}", tag="ps")
                for h in range(HG):
                    for nh in range(2):
                        nc.tensor.matmul(
                            pr_ps[:, nh * 512 : (nh + 1) * 512],
                            lhsT=ctx_sb[:, h, ic * 128 : (ic + 1) * 128],
                            rhs=wp_sb[:, h, nh * 512 : (nh + 1) * 512],
                            start=(h == 0),
                            stop=(h == HG - 1),
                        )
                out_sb = opool.tile([128, DIM], f32, name="out_sb", tag="out")
                if ic % 2 == 0:
                    nc.vector.tensor_copy(out=out_sb, in_=pr_ps)
                else:
                    nc.scalar.copy(out=out_sb, in_=pr_ps)
                nc.sync.dma_start(
                    out=out_d[ic * 128 : (ic + 1) * 128, :], in_=out_sb
                )

    nc.compile()
    return nc


def get_module():
    if "nc" not in _CACHE:
        _CACHE["nc"] = _build_module()
    return _CACHE["nc"]


def make_in_maps(x, mem, W_kv, W_q, W_proj):
    """Host-side shard + repack into the k-major fp32r layouts."""
    x = np.ascontiguousarray(np.asarray(x, np.float32))
    mem = np.ascontiguousarray(np.asarray(mem, np.float32))
    W_kv = np.asarray(W_kv, np.float32)
    W_q = np.asarray(W_q, np.float32)
    W_proj = np.asarray(W_proj, np.float32)

    def pack_k(a):  # [1024, N] -> [128, 8, N], k-chunked, fp32r-rounded
        n = a.shape[1]
        return round_fp32r(
            np.ascontiguousarray(a.reshape(KC, 128, n).transpose(1, 0, 2))
        )

    xt_b = [pack_k(x[b].T) for b in range(B)]
    memt_b = [pack_k(mem[b].T) for b in range(B)]
    ones = np.ones((1, HD), np.float32)
    vones = np.ones((128, JC * HG), np.float32)

    in_maps = []
    for core in range(N_CORES):
        b, g = divmod(core, 4)
        cs = slice(g * DD, (g + 1) * DD)
        wq = pack_k(W_q[:, cs])
        wk = pack_k(W_kv[:, :DIM][:, cs])
        wv = pack_k(W_kv[:, DIM:][:, cs])
        # wp[d, h, n] = W_proj[g*256 + h*64 + d, n]  (bf16 for 1-cyc streaming)
        wp = (
            np.ascontiguousarray(
                W_proj[cs, :].reshape(HG, HD, DIM).transpose(1, 0, 2)
            )
            .astype(ml_dtypes.bfloat16)
        )
        in_maps.append(
            {
                "xt": xt_b[b],
                "memt": memt_b[b],
                "wq": wq,
                "wk": wk,
                "wv": wv,
                "wp": wp,
                "ones_in": ones,
                "vones": vones,
            }
        )
    return in_maps


def combine_outputs(partials, b_proj):
    """Sum the 4 row-parallel partials per batch, add bias."""
    b_proj = np.asarray(b_proj, np.float32)
    out = np.zeros((B, NQ, DIM), np.float32)
    for core in range(N_CORES):
        out[core // 4] += partials[core]
    out += b_proj[None, None, :]
    return out


def kernel(x, mem, W_kv, W_q, W_proj, b_proj):
    from concourse import bass_utils

    nc = get_module()
    in_maps = make_in_maps(x, mem, W_kv, W_q, W_proj)
    res = bass_utils.run_bass_kernel_spmd(
        nc, in_maps, core_ids=list(range(N_CORES))
    )
    partials = [res.results[c]["out"] for c in range(N_CORES)]
    return combine_outputs(partials, b_proj)



# revision 30
# speedup vs baseline: 1.2715x; 1.2715x over previous
"""Trainium2 Bass kernel: CrossAttention (B=2, Nq=1024, Nkv=2048, D=1024, H=16).

Sharding: 8 cores = 2 (batch) x 4 (head groups of 4 heads).  fp16 data path
(10-bit mantissa ~ fp32r precision, half the DMA, full PE rate, FWL weight
loads).  Per core, for batch b and heads [4g, 4g+4):

    qT = (x_b @ Wq_s)^T          [128, mc=2, 1024]
    kT = (mem_b @ Wk_s)^T        [128, mc=2, 2048]
    v  = mem_b @ Wv_s  -> vaug   [128, jc=16, h=4, 65]  (ones column)
    per head pair hp = (2hp, 2hp+1), QK concurrent via PE row groups:
      sT = k_h @ q_h^T           [128, 1024] psum
      eT = exp(SCALE*sT)  fp16   (ACT; the only ACT function -> 1 table load)
      cu[65,1024] += [v|1]^T @ eT   (row 64 = softmax denominator)
      inv = recip_approx_fast(cu[64])  (DVE, no ACT table swap)
      binv = ones^T x inv        (PE broadcast; even head rows 0:64, odd 64:128)
      ctxp[:, hp, :] = cu[0:64] * binv  -> fp16, odd head at partitions 64:128
    proj: per ic: sum_hp ctxp[:, hp, ic]^T @ wp[:, hp, :]   (K=128 packed)

AV matmuls lag QK/exp by one jc chunk so the PE never stalls on ACT; the
denominator epilogue uses only DVE + 4 tiny MMs so the PE stays busy across
the hp boundary (keeps the HAM clock-gate at 8/8).
Host sums the 4 row-parallel fp16 partials per batch and adds b_proj.
"""

import numpy as np

DIM = 1024
HEADS = 16
HD = 64
B = 2
NQ = 1024
NKV = 2048
SCALE = HD ** -0.5
N_CORES = 8
HG = 4               # heads per core
DD = HG * HD         # 256 packed head dims per core
KC = 8               # contraction chunks (DIM / 128)
JC = NKV // 128      # 16 kv-row chunks

_CACHE = {}


def _build_module():
    import concourse.bacc as bacc
    import concourse.tile as tile
    import concourse.mybir as mybir

    f32 = mybir.dt.float32
    f16 = mybir.dt.float16
    EXP = mybir.ActivationFunctionType.Exp

    nc = bacc.Bacc(
        trn_type="TRN2",
        target_bir_lowering=False,
        debug=False,
        num_devices=N_CORES,
    )

    xt_d = nc.dram_tensor("xt", [128, KC, NQ], f16, kind="ExternalInput").ap()
    memt_d = nc.dram_tensor("memt", [128, KC, NKV], f16, kind="ExternalInput").ap()
    wq_d = nc.dram_tensor("wq", [128, KC, DD], f16, kind="ExternalInput").ap()
    wk_d = nc.dram_tensor("wk", [128, KC, DD], f16, kind="ExternalInput").ap()
    wv_d = nc.dram_tensor("wv", [128, KC, DD], f16, kind="ExternalInput").ap()
    wp_d = nc.dram_tensor("wp", [64, HG, DIM], f16, kind="ExternalInput").ap()
    ones_d = nc.dram_tensor("ones_in", [1, HD], f16, kind="ExternalInput").ap()
    vones_d = nc.dram_tensor("vones", [128, JC * HG], f16, kind="ExternalInput").ap()
    out_d = nc.dram_tensor("out", [NQ, DIM], f16, kind="ExternalOutput").ap()

    with tile.TileContext(nc) as tc:
        with (
            tc.tile_pool(name="wpool", bufs=1) as wpool,
            tc.tile_pool(name="persist", bufs=1) as persist,
            tc.tile_pool(name="xstream", bufs=2) as xstream,
            tc.tile_pool(name="work", bufs=2) as work,
            tc.tile_pool(name="opool", bufs=2) as opool,
            tc.tile_pool(name="psum", bufs=4, space="PSUM") as psum,
        ):
            # ---- qT inputs first so their DMAs lead the queues ----
            wq_sb = wpool.tile([128, KC, DD], f16, name="wq_sb")
            nc.sync.dma_start(out=wq_sb, in_=wq_d)

            # ---- qT projection: qT[dd, i] = sum_k Wq[k, dd] * x[i, k] ----
            qT_sb = persist.tile([128, 2, NQ], f16, name="qT_sb")
            qt_ps = [
                psum.tile([128, NQ], f32, name=f"qt_ps{mc}", tag="ps")
                for mc in range(2)
            ]
            for kc2 in range(KC // 2):
                xt_sb = xstream.tile([128, 2, NQ], f16, name="xt_sb", tag="xt")
                nc.sync.dma_start(out=xt_sb, in_=xt_d[:, 2 * kc2 : 2 * kc2 + 2, :])
                for dk in range(2):
                    kc = 2 * kc2 + dk
                    for mc in range(2):
                        for ih in range(2):
                            nc.tensor.matmul(
                                qt_ps[mc][:, ih * 512 : (ih + 1) * 512],
                                lhsT=wq_sb[:, kc, mc * 128 : (mc + 1) * 128],
                                rhs=xt_sb[:, dk, ih * 512 : (ih + 1) * 512],
                                start=(kc == 0),
                                stop=(kc == KC - 1),
                            )
            for mc in range(2):
                nc.vector.tensor_copy(out=qT_sb[:, mc, :], in_=qt_ps[mc])

            # remaining inputs
            wk_sb = wpool.tile([128, KC, DD], f16, name="wk_sb")
            nc.sync.dma_start(out=wk_sb, in_=wk_d)
            vones_sb = wpool.tile([128, JC * HG], f16, name="vones_sb")
            nc.sync.dma_start(out=vones_sb, in_=vones_d)
            ones_sb = wpool.tile([65, HD], f16, name="ones_sb")
            nc.sync.dma_start(out=ones_sb[64:65, :], in_=ones_d)
            memt_sb = []
            for kc2 in range(KC // 2):
                m = wpool.tile(
                    [128, 2, NKV], f16, name=f"memt_sb{kc2}", tag=f"memt{kc2}"
                )
                nc.sync.dma_start(out=m, in_=memt_d[:, 2 * kc2 : 2 * kc2 + 2, :])
                memt_sb.append(m[:, 0, :])
                memt_sb.append(m[:, 1, :])
            wv_sb = wpool.tile([128, KC, DD], f16, name="wv_sb")
            nc.sync.dma_start(out=wv_sb, in_=wv_d)
            wp_sb = wpool.tile([64, HG, DIM], f16, name="wp_sb")
            nc.sync.dma_start(out=wp_sb, in_=wp_d)

            # ---- persistent intermediates ----
            kT_sb = persist.tile([128, 2, NKV], f16, name="kT_sb")
            vaug_sb = persist.tile([128, JC, HG, HD + 1], f16, name="vaug_sb")
            ctx_sb = persist.tile([64, HG, NQ], f16, name="ctx_sb")

            nc.vector.tensor_copy(
                out=vaug_sb[:, :, :, HD : HD + 1],
                in_=vones_sb.rearrange("p (j h) -> p j h", j=JC)[:, :, :, None],
            )

            # ---- kT projection (consumes memt chunks as they arrive) ----
            kt_ps = {}
            for mc in range(2):
                for jh2 in range(2):
                    kt_ps[mc, jh2] = psum.tile(
                        [128, NKV // 2], f32, name=f"kt_ps_{mc}_{jh2}", tag="ps"
                    )
            for kc in range(KC):
                for mc in range(2):
                    for jh2 in range(2):
                        for jh in range(2):
                            j0 = (jh2 * 2 + jh) * 512
                            nc.tensor.matmul(
                                kt_ps[mc, jh2][:, jh * 512 : (jh + 1) * 512],
                                lhsT=wk_sb[:, kc, mc * 128 : (mc + 1) * 128],
                                rhs=memt_sb[kc][:, j0 : j0 + 512],
                                start=(kc == 0),
                                stop=(kc == KC - 1),
                            )
            for mc in range(2):
                for jh2 in range(2):
                    nc.vector.tensor_copy(
                        out=kT_sb[:, mc, jh2 * 1024 : (jh2 + 1) * 1024],
                        in_=kt_ps[mc, jh2],
                    )

            # ---- v projection: v[j, dd] = sum_k mem[j, k] * Wv[k, dd] ----
            for jc in range(JC):
                v_ps = psum.tile([128, DD], f32, name=f"v_ps{jc}", tag="ps")
                for kc in range(KC):
                    nc.tensor.matmul(
                        v_ps,
                        lhsT=memt_sb[kc][:, jc * 128 : (jc + 1) * 128],
                        rhs=wv_sb[:, kc, :],
                        start=(kc == 0),
                        stop=(kc == KC - 1),
                    )
                nc.vector.tensor_copy(
                    out=vaug_sb[:, jc, :, 0:HD],
                    in_=v_ps.rearrange("p (h d) -> p h d", h=HG),
                )

            # ---- attention, head pairs; AV lags QK/exp by one jc; the
            # denominator epilogue of pair hp is emitted after the first two
            # QK/exp blocks of pair hp+1 so the PE never idles long enough
            # for the HAM clock-gate to re-throttle.
            LN = mybir.ActivationFunctionType.Ln

            def cu_to_sbuf(heads, cu):
                # Drain the AV accumulators to SBUF right after the last AV so
                # their PSUM slots free early (next pair's QK can proceed while
                # the denominator chain runs from SBUF).
                cus = work.tile([65, 2, NQ], f32, name="cus_sb", tag="cus", bufs=2)
                for h in heads:
                    nc.vector.tensor_copy(out=cus[:, h % 2, :], in_=cu[h])
                return cus

            def epilogue(heads, cus):
                # 1/den = exp(-ln(den)) on ACT (fp16 out), PE row-broadcast.
                inv32 = work.tile([65, 2, NQ], f32, name="inv32", tag="inv32", bufs=2)
                inv16 = work.tile([65, 2, NQ], f16, name="inv16", tag="inv16", bufs=2)
                for h in heads:
                    nc.scalar.activation(
                        out=inv32[64:65, h % 2, :], in_=cus[64:65, h % 2, :], func=LN
                    )
                nc.scalar.activation(
                    out=inv16[64:65, :, :],
                    in_=inv32[64:65, :, :],
                    func=EXP,
                    scale=-1.0,
                )
                for h in heads:
                    binv_ps = psum.tile([64, NQ], f32, name=f"binv_ps{h}", tag="ps")
                    for ih in range(2):
                        nc.tensor.matmul(
                            binv_ps[:, ih * 512 : (ih + 1) * 512],
                            lhsT=ones_sb[64:65, :],
                            rhs=inv16[64:65, h % 2, ih * 512 : (ih + 1) * 512],
                            start=True,
                            stop=True,
                        )
                    binv_sb = work.tile(
                        [64, NQ], f32, name="binv_sb", tag="binv", bufs=2
                    )
                    nc.vector.tensor_copy(out=binv_sb, in_=binv_ps)
                    nc.vector.tensor_mul(
                        ctx_sb[:, h, :], cus[0:HD, h % 2, :], binv_sb
                    )

            pending = None
            for hp in range(2):
                heads = (2 * hp, 2 * hp + 1)
                cu = {
                    h: psum.tile([HD + 1, NQ], f32, name=f"cu_ps{h}", tag="ps")
                    for h in heads
                }
                prev = None
                for jc in range(JC):
                    sT = {}
                    eT = {}
                    for h in heads:
                        po = (h % 2) * 64
                        sT[h] = psum.tile(
                            [128, NQ], f32, name=f"sT_ps_{h}_{jc}", tag="ps"
                        )
                        for ih in range(2):
                            nc.tensor.matmul(
                                sT[h][:, ih * 512 : (ih + 1) * 512],
                                lhsT=kT_sb[
                                    po : po + 64, hp, jc * 128 : (jc + 1) * 128
                                ],
                                rhs=qT_sb[po : po + 64, hp, ih * 512 : (ih + 1) * 512],
                                start=True,
                                stop=True,
                            )
                    for h in heads:
                        e = work.tile([128, NQ], f16, name="eT_sb", tag="eT", bufs=4)
                        nc.scalar.activation(out=e, in_=sT[h], func=EXP, scale=SCALE)
                        eT[h] = e
                    if jc == 1 and pending is not None:
                        ph, pcus = pending
                        epilogue(ph, pcus)
                        pending = None
                    if prev is not None:
                        pjc, peT = prev
                        for h in heads:
                            for ih in range(2):
                                nc.tensor.matmul(
                                    cu[h][:, ih * 512 : (ih + 1) * 512],
                                    lhsT=vaug_sb[:, pjc, h, :],
                                    rhs=peT[h][:, ih * 512 : (ih + 1) * 512],
                                    start=(pjc == 0),
                                    stop=False,
                                )
                    prev = (jc, eT)
                pjc, peT = prev
                for h in heads:
                    for ih in range(2):
                        nc.tensor.matmul(
                            cu[h][:, ih * 512 : (ih + 1) * 512],
                            lhsT=vaug_sb[:, pjc, h, :],
                            rhs=peT[h][:, ih * 512 : (ih + 1) * 512],
                            start=False,
                            stop=True,
                        )
                pending = (heads, cu_to_sbuf(heads, cu))

            # ---- output projection: K=64 per head, accumulate over heads.
            # Heads 0/1 of ic 0-1 are emitted before the last epilogue so the
            # PE has work while ACT/DVE finish the hp1 denominators.
            def proj_mms(pr_ps, ic, hs, start, stop):
                for h in hs:
                    for nh in range(2):
                        nc.tensor.matmul(
                            pr_ps[:, nh * 512 : (nh + 1) * 512],
                            lhsT=ctx_sb[:, h, ic * 128 : (ic + 1) * 128],
                            rhs=wp_sb[:, h, nh * 512 : (nh + 1) * 512],
                            start=start and h == hs[0],
                            stop=stop and h == hs[-1],
                        )

            pr_head = {}
            for ic in range(2):
                pr_head[ic] = psum.tile([128, DIM], f32, name=f"pr_ps{ic}", tag="ps")
                proj_mms(pr_head[ic], ic, [0, 1], True, False)

            ph, pcus = pending
            epilogue(ph, pcus)

            def proj_finish(pr_ps, ic):
                out_sb = opool.tile([128, DIM], f16, name="out_sb", tag="out")
                if ic % 2 == 0:
                    nc.vector.tensor_copy(out=out_sb, in_=pr_ps)
                else:
                    nc.scalar.copy(out=out_sb, in_=pr_ps)
                nc.sync.dma_start(out=out_d[ic * 128 : (ic + 1) * 128, :], in_=out_sb)

            for ic in range(2):
                proj_mms(pr_head[ic], ic, [2, 3], False, True)
                proj_finish(pr_head[ic], ic)
            for ic in range(2, 8):
                pr_ps = psum.tile([128, DIM], f32, name=f"pr_ps{ic}", tag="ps")
                proj_mms(pr_ps, ic, [0, 1, 2, 3], True, True)
                proj_finish(pr_ps, ic)

    nc.compile()
    return nc


def get_module():
    if "nc" not in _CACHE:
        _CACHE["nc"] = _build_module()
    return _CACHE["nc"]


def make_in_maps(x, mem, W_kv, W_q, W_proj):
    """Host-side shard + repack into k-major fp16 layouts."""
    x = np.ascontiguousarray(np.asarray(x, np.float32))
    mem = np.ascontiguousarray(np.asarray(mem, np.float32))
    W_kv = np.asarray(W_kv, np.float32)
    W_q = np.asarray(W_q, np.float32)
    W_proj = np.asarray(W_proj, np.float32)

    def pack_k(a):  # [1024, N] -> [128, KC, N] fp16
        n = a.shape[1]
        return np.ascontiguousarray(
            a.reshape(KC, 128, n).transpose(1, 0, 2)
        ).astype(np.float16)

    xt_b = [pack_k(x[b].T) for b in range(B)]
    memt_b = [pack_k(mem[b].T) for b in range(B)]
    ones = np.ones((1, HD), np.float16)
    vones = np.ones((128, JC * HG), np.float16)

    in_maps = []
    for core in range(N_CORES):
        b, g = divmod(core, 4)
        cs = slice(g * DD, (g + 1) * DD)
        wq = pack_k(W_q[:, cs])
        wk = pack_k(W_kv[:, :DIM][:, cs])
        wv = pack_k(W_kv[:, DIM:][:, cs])
        # wp[d, h, n] = W_proj[g*256 + h*64 + d, n]
        wp = np.ascontiguousarray(
            W_proj[cs, :].reshape(HG, HD, DIM).transpose(1, 0, 2)
        ).astype(np.float16)
        in_maps.append(
            {
                "xt": xt_b[b],
                "memt": memt_b[b],
                "wq": wq,
                "wk": wk,
                "wv": wv,
                "wp": wp,
                "ones_in": ones,
                "vones": vones,
            }
        )
    return in_maps


def combine_outputs(partials, b_proj):
    """Sum the 4 row-parallel partials per batch, add bias."""
    b_proj = np.asarray(b_proj, np.float32)
    out = np.zeros((B, NQ, DIM), np.float32)
    for core in range(N_CORES):
        out[core // 4] += np.asarray(partials[core], np.float32)
    out += b_proj[None, None, :]
    return out


def kernel(x, mem, W_kv, W_q, W_proj, b_proj):
    from concourse import bass_utils

    nc = get_module()
    in_maps = make_in_maps(x, mem, W_kv, W_q, W_proj)
    res = bass_utils.run_bass_kernel_spmd(
        nc, in_maps, core_ids=list(range(N_CORES))
    )
    partials = [res.results[c]["out"] for c in range(N_CORES)]
    return combine_outputs(partials, b_proj)
